# revision 5
# baseline (speedup 1.0000x reference)
"""Trainium2 Bass kernel for nn_KANLayer (piecewise-constant KAN forward).

Math: reference computes out[t,i] = sum_j f[i,j,m(x_tj)] where m = segment(x)
in 0..8 and f[i,j,m] = c_m + c_{m+1} + c_{m+2} (9-valued selection). The whole
contraction runs in fp8-e4m3 DoubleRow (K=256 per 213ns N=512 matmul):

    out[t,i] = base_i + sum_a co_a[i,j]*phi_a(m_tj)   2 eigen-planes, 4 units
             + sum_{m!=4} R[i,j,m] * onehot_m(t,j)    8 planes, 16 units

phi1/phi2 are the top-2 eigenvectors of the (m!=4) covariance of
D = f - f(4), snapped to fp8-exact values (they capture ~83%% of residual
variance vs ~64%% for the lin/quad pair, host-sim rel err 1.38e-2 vs 1.83e-2).
Tables are quantized scale-free (fp8 is floating point, per-row scaling buys
nothing), with co quantized first so its error is absorbed by the
later-quantized one-hot residual R; residual at m=4 is exactly zero (base
anchored at f(4)), so the m=4 plane is dropped. The m7/m8 one-hot planes are
device-built (DVE bf16 is_equal on the phi2 plane -- its snapped values are
kept distinct -- then ACT copy-convert to fp8); everything else ships as raw
e4m3 bytes from host.

Schedule per core: 20 units x 4 out-blocks x 4 token-groups = 320 DR matmuls
(213ns each, ~68us PE). Every (ob, tg) accumulator splits into gen-A (12
early-DMA units) and gen-B (8 late units: m6 + phi2 + device-built m7/m8).
gen-A partials spill to SBUF f32 with the output bias pre-added (one DVE op),
so the PE has 16 real work streams on 8 PSUM banks and needs no warmup spin
while input DMA ramps. Planes ship as token-half pieces ([128,p,2,JC,1024]
layout) so gen-A's first half only waits on half the plane bytes. gen-B runs
accumulator-outer so final evacuations (one DVE tensor_tensor add each)
stagger ~1.7us apart instead of bursting after the last matmul; the last
slice evacuates in two pipelined halves. Output leaves as
[out_block, 128i, tok] bf16, upcast/transposed on host. Sharding:
data-parallel over tokens, 2048 per core; tables replicated.
"""

from contextlib import ExitStack

import numpy as np

import concourse.bass as bass  # noqa: F401
import concourse.tile as tile
from concourse import bacc, mybir
from concourse.bass_utils import run_bass_kernel_spmd

N_CORES = 8
TOK = 2048          # tokens per core
HTOK = TOK // 2
IN_F = 512
OUT_F = 512
JC = IN_F // 128    # 4 j-chunks of 128
NPASS = OUT_F // 128  # 4 out-blocks
NTG = 4             # token groups (N=512 matmuls) per out-block
TGW = TOK // NTG
NU = 20             # DR units: 4 eigen-affine + 16 one-hot (m!=4)
NSHIP = 8           # shipped planes: phi1, phi2, oh m0..m3, m5, m6
FP8 = mybir.dt.float8e4
BF16 = mybir.dt.bfloat16
F32 = mybir.dt.float32
E4NP = mybir.dt.np(FP8)  # ml_dtypes.float8_e4m3 (TRN: bias 7, max 240)

# plane slots in pl_t: 0=phi1, 1=phi2, 2..7=onehot m0,m1,m2,m3,m5,m6,
# 8=onehot m7 (device), 9=onehot m8 (device)
OH_MS = [0, 1, 2, 3, 5, 6]          # shipped one-hot m values (slots 2..7)

# unit -> (plane slot, jc-pair q), in PE stream order.
# gen-A (12): phi1, oh m0..m3, m5.  gen-B (8): oh m6, phi2, oh m7, oh m8.
UA = 12
_UNIT_PKS = [0, 2, 3, 4, 5, 6, 7, 1, 8, 9]
_UNITS = [(pk, q) for pk in _UNIT_PKS for q in range(2)]
assert len(_UNITS) == NU

_PROGRAM_CACHE = {}


def _build_program():
    nc = bacc.Bacc("TRN2", target_bir_lowering=False, debug=False)

    pl_d = nc.dram_tensor("pl", [128, NSHIP, 2, JC, HTOK], FP8,
                          kind="ExternalInput").ap()
    g_d = nc.dram_tensor("g", [128, NU, 2, NPASS, 128], FP8,
                         kind="ExternalInput").ap()
    sb_d = nc.dram_tensor("sb", [128, NPASS], F32, kind="ExternalInput").ap()
    out_d = nc.dram_tensor("out", [NPASS, 128, TOK], BF16,
                           kind="ExternalOutput").ap()

    with tile.TileContext(nc) as tc, ExitStack() as ctx:
        tmp_pool = ctx.enter_context(tc.tile_pool(name="tmp", bufs=2))
        pl_pool = ctx.enter_context(tc.tile_pool(name="pl", bufs=1))
        g_pool = ctx.enter_context(tc.tile_pool(name="g", bufs=1))
        sb_pool = ctx.enter_context(tc.tile_pool(name="sb", bufs=1))
        part_pool = ctx.enter_context(tc.tile_pool(name="part", bufs=1))
        out_pool = ctx.enter_context(tc.tile_pool(name="out", bufs=4))
        psum_pool = ctx.enter_context(tc.tile_pool(name="psum", bufs=8,
                                                   space="PSUM"))

        # --- input DMAs, deadline-ordered across the two HWDGE rings.
        # Token-half plane pieces (524KB each): h=0 halves + their tables
        # first (gen-A h=0 stream), phi2 early so ACT can build m7/m8 planes,
        # h=1 halves after, m6 (gen-B) last.
        pl_t = pl_pool.tile([128, NSHIP + 2, 2, JC, HTOK], FP8, name="pl")
        g_t = g_pool.tile([128, NU, 2, NPASS, 128], FP8, name="g")
        sb_t = sb_pool.tile([128, NPASS], F32, name="sb")
        nc.sync.dma_start(pl_t[:, 0, 0], pl_d[:, 0, 0])      # phi1 h0
        nc.scalar.dma_start(g_t[:, 0:2], g_d[:, 0:2])        # phi1 tables
        nc.gpsimd.dma_start(sb_t[:], sb_d[:])
        nc.sync.dma_start(pl_t[:, 2, 0], pl_d[:, 2, 0])      # oh m0 h0
        nc.scalar.dma_start(g_t[:, 2:6], g_d[:, 2:6])
        nc.sync.dma_start(pl_t[:, 3, 0], pl_d[:, 3, 0])      # oh m1 h0
        nc.scalar.dma_start(pl_t[:, 4, 0], pl_d[:, 4, 0])    # oh m2 h0
        nc.sync.dma_start(pl_t[:, 1, 0], pl_d[:, 1, 0])      # phi2 h0 (ACT)
        nc.scalar.dma_start(g_t[:, 6:10], g_d[:, 6:10])
        nc.sync.dma_start(pl_t[:, 5, 0], pl_d[:, 5, 0])      # oh m3 h0
        nc.scalar.dma_start(pl_t[:, 1, 1], pl_d[:, 1, 1])    # phi2 h1 (ACT)
        nc.sync.dma_start(pl_t[:, 6, 0], pl_d[:, 6, 0])      # oh m5 h0
        nc.scalar.dma_start(g_t[:, 10:14], g_d[:, 10:14])
        nc.sync.dma_start(pl_t[:, 0, 1], pl_d[:, 0, 1])      # phi1 h1
        nc.scalar.dma_start(pl_t[:, 2, 1], pl_d[:, 2, 1])    # oh m0 h1
        nc.sync.dma_start(pl_t[:, 3, 1], pl_d[:, 3, 1])      # oh m1 h1
        nc.scalar.dma_start(g_t[:, 14:20], g_d[:, 14:20])
        nc.sync.dma_start(pl_t[:, 4, 1], pl_d[:, 4, 1])      # oh m2 h1
        nc.scalar.dma_start(pl_t[:, 5, 1], pl_d[:, 5, 1])    # oh m3 h1
        nc.sync.dma_start(pl_t[:, 6, 1], pl_d[:, 6, 1])      # oh m5 h1
        nc.scalar.dma_start(pl_t[:, 7, 0], pl_d[:, 7, 0])    # oh m6 h0
        nc.sync.dma_start(pl_t[:, 7, 1], pl_d[:, 7, 1])      # oh m6 h1

        # Device-built planes: one-hot m7/m8 via DVE bf16 is_equal on the
        # phi2 plane (values kept distinct host-side) + ACT copy-convert to
        # fp8, per (plane, h, jc-pair) chunk.
        for slot, mval in ((8, 7), (9, 8)):
            for h in range(2):
                tmp = tmp_pool.tile([128, JC, HTOK], BF16, name="ohb")
                nc.vector.tensor_scalar(
                    tmp[:], pl_t[:, 1, h],
                    _PROGRAM_CACHE["phi2_cmp"][mval], None,
                    mybir.AluOpType.is_equal,
                )
                for q in range(2):
                    nc.scalar.activation(
                        pl_t[:, slot, h, 2 * q:2 * q + 2],
                        tmp[:, 2 * q:2 * q + 2],
                        mybir.ActivationFunctionType.Copy,
                    )

        def mm(ps, ob, u, tg, start, stop):
            pk, q = _UNITS[u]
            h, off = tg // 2, (tg % 2) * TGW
            nc.tensor.matmul(
                ps,
                g_t[:, u, :, ob, :],
                pl_t[:, pk, h, 2 * q:2 * q + 2, off:off + TGW],
                start=start,
                stop=stop,
                perf_mode=mybir.MatmulPerfMode.DoubleRow,
            )

        # part_t[:, h, ob, tgi*TGW:...] = gen-A partial + bias for (ob, tg)
        part_t = part_pool.tile([128, 2, NPASS, TGW * 2], F32, name="part")

        # gen-A: unit-outer (DMA arrival order), tg halves h=0 then h=1.
        pss = {}
        for h in (0, 1):
            for u in range(UA):
                for ob in range(NPASS):
                    for tgi in (0, 1):
                        if u == 0:
                            pss[ob, tgi] = psum_pool.tile(
                                [128, TGW], F32, name="ps")
                        mm(pss[ob, tgi][:], ob, u, 2 * h + tgi,
                           start=(u == 0), stop=(u == UA - 1))
            for ob in range(NPASS):
                for tgi in (0, 1):
                    nc.vector.tensor_scalar(
                        part_t[:, h, ob, tgi * TGW:(tgi + 1) * TGW],
                        pss[ob, tgi][:], sb_t[:, ob:ob + 1], None,
                        mybir.AluOpType.add,
                    )

        # gen-B: accumulator-outer so evacs stagger; one DVE add each.
        for h in (0, 1):
            for ob in range(NPASS):
                for tgi in (0, 1):
                    tg = 2 * h + tgi
                    ps = psum_pool.tile([128, TGW], F32, name="ps")
                    for ui, u in enumerate(range(UA, NU)):
                        mm(ps[:], ob, u, tg,
                           start=(ui == 0), stop=(u == NU - 1))
                    eng = nc.sync if (ob + tgi) % 2 == 0 else nc.scalar
                    last = (h == 1 and ob == NPASS - 1 and tgi == 1)
                    nhalf = 2 if last else 1
                    hw = TGW // nhalf
                    for hh in range(nhalf):
                        ot = out_pool.tile([128, hw], BF16, name="ot")
                        nc.vector.tensor_tensor(
                            ot[:], ps[:, hh * hw:(hh + 1) * hw],
                            part_t[:, h, ob, tgi * TGW + hh * hw:
                                   tgi * TGW + (hh + 1) * hw],
                            mybir.AluOpType.add,
                        )
                        eng.dma_start(
                            out_d[ob][:, tg * TGW + hh * hw:
                                      tg * TGW + (hh + 1) * hw],
                            ot[:],
                        )

    nc.compile()
    return nc


def _get_program(phi2_cmp=None):
    # phi2 compare constants are baked into the program; rebuild if they
    # change (same coeffs -> same program).
    if phi2_cmp is None:
        return _PROGRAM_CACHE["nc"]
    key = ("nc", tuple(sorted(phi2_cmp.items())))
    if _PROGRAM_CACHE.get("key") != key:
        _PROGRAM_CACHE["phi2_cmp"] = phi2_cmp
        _PROGRAM_CACHE["nc"] = _build_program()
        _PROGRAM_CACHE["key"] = key
    return _PROGRAM_CACHE["nc"]


def _plane_dev(arr):
    """[T_all, IN] -> [128, 2, JC, T_all/2-per-core] device half-layout.

    j = jc*128 + p; token axis split per-core later. Returns
    [128, JC, T_all] view to be sliced per core then half-split."""
    return np.ascontiguousarray(arr.T.reshape(JC, 128, -1).transpose(1, 0, 2))


def _pack_pair(tab_b):
    """e4m3 [OUT, IN] -> [128p, 2q, 2e, NPASS, 128col] stationary layout."""
    t = tab_b.reshape(NPASS, 128, JC, 128).transpose(3, 2, 0, 1)
    return np.ascontiguousarray(t.reshape(128, 2, 2, NPASS, 128))


def _snap_phi(phi, grid):
    """Snap phi (phi[4]=0 preserved) to fp8-exact values, scaled to ~12."""
    ph = (phi * (12.0 / np.abs(phi).max())).astype(E4NP).astype(np.float64)
    ph[4] = 0.0
    return ph


def _fp8_grid():
    b = np.arange(256, dtype=np.uint8).view(E4NP).astype(np.float64)
    vals = np.unique(b[np.isfinite(b)])
    return vals


def kernel(x: np.ndarray, coeffs: np.ndarray) -> np.ndarray:
    assert x.shape == (8, 2048, IN_F) and coeffs.shape == (OUT_F, IN_F, 12)
    t = np.linspace(0.0, 1.0, 10, dtype=np.float32)  # same knots as reference

    # Segment index via the same float32 comparisons the reference uses.
    xf = np.ascontiguousarray(x.reshape(-1, IN_F))          # [16384, 512]
    seg = np.zeros(xf.shape, dtype=np.int32)
    for m in range(1, 9):
        seg += (xf >= t[m]).astype(np.int32)

    # Table build (see module docstring): scale-free e4m3; phi1/phi2 = top
    # eigenvectors of the m!=4 covariance, fp8-snapped (phi2 values kept
    # distinct for the device is_equal builds); co quantized first
    # (absorbed), residual quantized last, res[4] pinned 0.
    c = coeffs.astype(np.float64)
    F = np.stack(
        [c[:, :, m] + c[:, :, m + 1] + c[:, :, m + 2] for m in range(9)]
    ).reshape(9, -1)                                         # [9, OUT*IN]
    D = F - F[4:5]
    idx = [0, 1, 2, 3, 5, 6, 7, 8]
    C8 = (D[idx] @ D[idx].T) / D.shape[1]
    _, V = np.linalg.eigh(C8)
    grid = _fp8_grid()
    phis = []
    for k in (-1, -2):
        ph = np.zeros(9)
        ph[idx] = V[:, k]
        phis.append(_snap_phi(ph, grid))
    phi1, phi2 = phis
    # ensure phi2 values at m=7,8 are unique (needed for is_equal builds)
    for m in (7, 8):
        others = set(np.delete(phi2, m).tolist())
        if phi2[m] in others:
            gi = int(np.searchsorted(grid, phi2[m]))
            for step in (1, -1, 2, -2, 3, -3):
                cand = grid[(gi + step) % len(grid)]
                if cand not in others and cand != 0.0:
                    phi2[m] = cand
                    break
    assert len(set(phi2[idx].tolist())) == len(idx)

    Phi = np.stack([phi1, phi2], axis=1)                     # [9, 2]
    co = np.linalg.lstsq(Phi[idx], D[idx], rcond=None)[0]    # [2, OUT*IN]

    def q8(v):
        return np.clip(v, -240.0, 240.0).astype(E4NP)

    co1b = q8(co[0].reshape(OUT_F, IN_F))
    co2b = q8(co[1].reshape(OUT_F, IN_F))
    res = D.reshape(9, OUT_F, IN_F) \
        - co1b.astype(np.float64)[None] * phi1[:, None, None] \
        - co2b.astype(np.float64)[None] * phi2[:, None, None]
    Rb = q8(res)
    Rb[4] = 0

    # g tables in unit order: phi1, oh m0..m3, m5 | m6, phi2, m7, m8
    tabs = [co1b] + [Rb[m] for m in (0, 1, 2, 3, 5)] \
        + [Rb[6], co2b, Rb[7], Rb[8]]
    g_dev = np.empty((128, NU, 2, NPASS, 128), dtype=E4NP)
    for ti, tab in enumerate(tabs):
        pk = _pack_pair(tab)
        for q in range(2):
            g_dev[:, 2 * ti + q] = pk[:, q]
    g_dev = np.ascontiguousarray(g_dev)

    base = F[4].reshape(OUT_F, IN_F).sum(axis=1)             # exact fp32
    sb = np.empty((128, NPASS), dtype=np.float32)
    for ob in range(NPASS):
        sb[:, ob] = base[ob * 128:(ob + 1) * 128]

    # Plane bytes via uint8 LUTs over seg (fast).
    planes = np.empty((128, NSHIP, JC, seg.shape[0]), dtype=E4NP)
    for slot, vals in ((0, phi1), (1, phi2)):
        lut = vals.astype(E4NP).view(np.uint8)
        planes[:, slot] = _plane_dev(lut[seg]).view(E4NP)
    for k, m in enumerate(OH_MS):
        lut = np.zeros(9, E4NP)
        lut[m] = 1.0
        planes[:, 2 + k] = _plane_dev(lut.view(np.uint8)[seg]).view(E4NP)

    in_maps = []
    for core in range(N_CORES):
        sl = planes[:, :, :, core * TOK:(core + 1) * TOK]
        # [128, NSHIP, JC, TOK] -> [128, NSHIP, 2, JC, HTOK] half-major
        halves = sl.reshape(128, NSHIP, JC, 2, HTOK).transpose(0, 1, 3, 2, 4)
        in_maps.append(
            {
                "pl": np.ascontiguousarray(halves),
                "g": g_dev,
                "sb": sb,
            }
        )

    phi2_cmp = {7: float(phi2[7]), 8: float(phi2[8])}
    nc = _get_program(phi2_cmp)
    res_ = run_bass_kernel_spmd(nc, in_maps, core_ids=list(range(N_CORES)))
    out = np.stack(
        [
            res_.results[core]["out"].reshape(OUT_F, TOK).T.astype(np.float32)
            for core in range(N_CORES)
        ]
    )
    return np.ascontiguousarray(out)


# revision 10
# speedup vs baseline: 1.1155x; 1.1155x over previous
"""Trainium2 Bass kernel for nn_KANLayer (piecewise-constant KAN forward).

Math: reference computes out[t,i] = sum_j f[i,j,m(x_tj)] where m = segment(x)
in 0..8 and f[i,j,m] = c_m + c_{m+1} + c_{m+2} (9-valued selection). The whole
contraction runs in fp8-e4m3 DoubleRow (K=256 per 213ns N=512 matmul):

    out[t,i] = base_i + sum_a co_a[i,j]*phi_a(m_tj)   2 eigen-planes, 4 units
             + sum_{m!=4} R[i,j,m] * onehot_m(t,j)    8 planes, 16 units

phi1/phi2 are the top-2 eigenvectors of the (m!=4) covariance of
D = f - f(4), snapped to fp8-exact values (they capture ~83%% of residual
variance vs ~64%% for the lin/quad pair, host-sim rel err 1.38e-2 vs 1.83e-2).
Tables are quantized scale-free (fp8 is floating point, per-row scaling buys
nothing), with co quantized first so its error is absorbed by the
later-quantized one-hot residual R; residual at m=4 is exactly zero (base
anchored at f(4)), so the m=4 plane is dropped. The m7/m8 one-hot planes are
device-built (DVE bf16 is_equal on the phi2 plane -- its snapped values are
kept distinct -- then ACT copy-convert to fp8); everything else ships as raw
e4m3 bytes from host.

Schedule per core: 20 units x 4 out-blocks x 4 token-groups = 320 DR matmuls
(213ns each, ~68us PE). Every (ob, tg) accumulator splits into gen-A (12
early-DMA units) and gen-B (8 late units: m6 + phi2 + device-built m7/m8).
gen-A partials spill to SBUF f32 with the output bias pre-added (one DVE op),
so the PE has 16 real work streams on 8 PSUM banks and needs no warmup spin
while input DMA ramps. Planes ship as token-half pieces ([128,p,2,JC,1024]
layout) so gen-A's first half only waits on half the plane bytes. gen-B runs
accumulator-outer so final evacuations (one DVE tensor_tensor add each)
stagger ~1.7us apart instead of bursting after the last matmul; the last
slice evacuates in two pipelined halves. Output leaves as
[out_block, 128i, tok] bf16, upcast/transposed on host. Sharding:
data-parallel over tokens, 2048 per core; tables replicated.
"""

from contextlib import ExitStack

import numpy as np

import concourse.bass as bass  # noqa: F401
import concourse.tile as tile
from concourse import bacc, mybir
from concourse.bass_utils import run_bass_kernel_spmd

N_CORES = 8
TOK = 2048          # tokens per core
HTOK = TOK // 2
IN_F = 512
OUT_F = 512
JC = IN_F // 128    # 4 j-chunks of 128
NPASS = OUT_F // 128  # 4 out-blocks
NTG = 4             # token groups (N=512 matmuls) per out-block
TGW = TOK // NTG
NU = 20             # DR units: 4 eigen-affine + 16 one-hot (m!=4)
NSHIP = 8           # shipped planes: phi1, phi2, oh m0..m3, m5, m6
FP8 = mybir.dt.float8e4
BF16 = mybir.dt.bfloat16
F32 = mybir.dt.float32
E4NP = mybir.dt.np(FP8)  # ml_dtypes.float8_e4m3 (TRN: bias 7, max 240)

# plane slots in pl_t: 0=phi1, 1=phi2, 2..7=onehot m0,m1,m2,m3,m5,m6,
# 8=onehot m7 (device), 9=onehot m8 (device)
OH_MS = [0, 1, 2, 3, 5, 6]          # shipped one-hot m values (slots 2..7)

# unit -> (plane slot, jc-pair q), in PE stream order.
# gen-A (12): phi1, oh m0..m3, m5.  gen-B (8): oh m6, phi2, oh m7, oh m8.
UA = 12
_UNIT_PKS = [0, 2, 3, 4, 5, 6, 7, 1, 8, 9]
_UNITS = [(pk, q) for pk in _UNIT_PKS for q in range(2)]
assert len(_UNITS) == NU

_PROGRAM_CACHE = {}


def _build_program():
    nc = bacc.Bacc("TRN2", target_bir_lowering=False, debug=False)

    pl_d = nc.dram_tensor("pl", [128, NSHIP, JC, TOK], FP8,
                          kind="ExternalInput").ap()
    g_d = nc.dram_tensor("g", [128, NU, 2, NPASS, 128], FP8,
                         kind="ExternalInput").ap()
    sb_d = nc.dram_tensor("sb", [128, NPASS], F32, kind="ExternalInput").ap()
    out_d = nc.dram_tensor("out", [NPASS, 128, TOK], BF16,
                           kind="ExternalOutput").ap()

    with tile.TileContext(nc) as tc, ExitStack() as ctx:
        tmp_pool = ctx.enter_context(tc.tile_pool(name="tmp", bufs=2))
        pl_pool = ctx.enter_context(tc.tile_pool(name="pl", bufs=1))
        g_pool = ctx.enter_context(tc.tile_pool(name="g", bufs=1))
        sb_pool = ctx.enter_context(tc.tile_pool(name="sb", bufs=1))
        part_pool = ctx.enter_context(tc.tile_pool(name="part", bufs=1))
        out_pool = ctx.enter_context(tc.tile_pool(name="out", bufs=4))
        psum_pool = ctx.enter_context(tc.tile_pool(name="psum", bufs=8,
                                                   space="PSUM"))

        # --- input DMAs: (plane, jc-pair) pieces (524KB, contiguous
        # 4KB/partition) in unit-stream order so unit (pk, q) waits only on
        # its own piece; first pieces spread over 4 queues to parallelize
        # the ring ramp; phi2 early so ACT can build the m7/m8 planes; m6
        # (gen-B) last.
        pl_t = pl_pool.tile([128, NSHIP + 2, JC, TOK], FP8, name="pl")
        g_t = g_pool.tile([128, NU, 2, NPASS, 128], FP8, name="g")
        sb_t = sb_pool.tile([128, NPASS], F32, name="sb")

        def pl_dma(eng, pk, q):
            eng.dma_start(pl_t[:, pk, 2 * q:2 * q + 2],
                          pl_d[:, pk, 2 * q:2 * q + 2])

        pl_dma(nc.sync, 0, 0)                                # phi1 q0
        nc.scalar.dma_start(g_t[:, 0:2], g_d[:, 0:2])        # phi1 tables
        pl_dma(nc.gpsimd, 0, 1)                              # phi1 q1
        nc.gpsimd.dma_start(sb_t[:], sb_d[:])
        pl_dma(nc.gpsimd, 2, 0)                              # oh m0 q0
        pl_dma(nc.sync, 2, 1)                                # oh m0 q1
        nc.scalar.dma_start(g_t[:, 2:6], g_d[:, 2:6])
        pl_dma(nc.sync, 3, 0)                                # oh m1 q0
        pl_dma(nc.scalar, 3, 1)                              # oh m1 q1
        pl_dma(nc.sync, 1, 0)                                # phi2 q0 (ACT)
        pl_dma(nc.scalar, 1, 1)                              # phi2 q1 (ACT)
        pl_dma(nc.sync, 4, 0)                                # oh m2 q0
        nc.scalar.dma_start(g_t[:, 6:10], g_d[:, 6:10])
        pl_dma(nc.sync, 4, 1)                                # oh m2 q1
        pl_dma(nc.scalar, 5, 0)                              # oh m3 q0
        pl_dma(nc.sync, 5, 1)                                # oh m3 q1
        nc.scalar.dma_start(g_t[:, 10:14], g_d[:, 10:14])
        pl_dma(nc.sync, 6, 0)                                # oh m5 q0
        pl_dma(nc.scalar, 6, 1)                              # oh m5 q1
        nc.scalar.dma_start(g_t[:, 14:20], g_d[:, 14:20])
        pl_dma(nc.sync, 7, 0)                                # oh m6 q0
        pl_dma(nc.scalar, 7, 1)                              # oh m6 q1

        # Device-built planes: one-hot m7/m8 via DVE bf16 is_equal on the
        # phi2 plane (values kept distinct host-side) + ACT copy-convert to
        # fp8, per jc-pair chunk.
        for slot, mval in ((8, 7), (9, 8)):
            for q in range(2):
                tmp = tmp_pool.tile([128, 2, TOK], BF16, name="ohb")
                nc.vector.tensor_scalar(
                    tmp[:], pl_t[:, 1, 2 * q:2 * q + 2],
                    _PROGRAM_CACHE["phi2_cmp"][mval], None,
                    mybir.AluOpType.is_equal,
                )
                nc.scalar.activation(
                    pl_t[:, slot, 2 * q:2 * q + 2],
                    tmp[:],
                    mybir.ActivationFunctionType.Copy,
                )

        def mm(ps, ob, u, tg, start, stop):
            pk, q = _UNITS[u]
            nc.tensor.matmul(
                ps,
                g_t[:, u, :, ob, :],
                pl_t[:, pk, 2 * q:2 * q + 2, tg * TGW:(tg + 1) * TGW],
                start=start,
                stop=stop,
                perf_mode=mybir.MatmulPerfMode.DoubleRow,
            )

        # part_t[:, h, ob, tgi*TGW:...] = gen-A partial + bias for (ob, tg)
        part_t = part_pool.tile([128, 2, NPASS, TGW * 2], F32, name="part")

        # gen-A: unit-outer (DMA arrival order), tg halves h=0 then h=1.
        pss = {}
        for h in (0, 1):
            for u in range(UA):
                for ob in range(NPASS):
                    for tgi in (0, 1):
                        if u == 0:
                            pss[ob, tgi] = psum_pool.tile(
                                [128, TGW], F32, name="ps")
                        mm(pss[ob, tgi][:], ob, u, 2 * h + tgi,
                           start=(u == 0), stop=(u == UA - 1))
            for ob in range(NPASS):
                for tgi in (0, 1):
                    nc.vector.tensor_scalar(
                        part_t[:, h, ob, tgi * TGW:(tgi + 1) * TGW],
                        pss[ob, tgi][:], sb_t[:, ob:ob + 1], None,
                        mybir.AluOpType.add,
                    )

        # gen-B: accumulator-outer so evacs stagger (one DVE add each); out
        # DMA batched per (h, ob) except the last pair, whose second half
        # evacuates in two pipelined quarters for the shortest exposed tail.
        for h in (0, 1):
            for ob in range(NPASS):
                otb = out_pool.tile([128, 2 * TGW], BF16, name="otb")
                last = (h == 1 and ob == NPASS - 1)
                eng = nc.sync if ob % 2 == 0 else nc.scalar
                for tgi in (0, 1):
                    tg = 2 * h + tgi
                    ps = psum_pool.tile([128, TGW], F32, name="ps")
                    for ui, u in enumerate(range(UA, NU)):
                        mm(ps[:], ob, u, tg,
                           start=(ui == 0), stop=(u == NU - 1))
                    nhalf = 2 if (last and tgi == 1) else 1
                    hw = TGW // nhalf
                    for hh in range(nhalf):
                        sl = slice(tgi * TGW + hh * hw,
                                   tgi * TGW + (hh + 1) * hw)
                        nc.vector.tensor_tensor(
                            otb[:, sl], ps[:, hh * hw:(hh + 1) * hw],
                            part_t[:, h, ob, sl],
                            mybir.AluOpType.add,
                        )
                        if last:
                            eng.dma_start(
                                out_d[ob][:, 2 * h * TGW:][:, sl],
                                otb[:, sl])
                if not last:
                    eng.dma_start(
                        out_d[ob][:, 2 * h * TGW:2 * (h + 1) * TGW], otb[:])

    nc.compile()
    return nc


def _get_program(phi2_cmp=None):
    # phi2 compare constants are baked into the program; rebuild if they
    # change (same coeffs -> same program).
    if phi2_cmp is None:
        return _PROGRAM_CACHE["nc"]
    key = ("nc", tuple(sorted(phi2_cmp.items())))
    if _PROGRAM_CACHE.get("key") != key:
        _PROGRAM_CACHE["phi2_cmp"] = phi2_cmp
        _PROGRAM_CACHE["nc"] = _build_program()
        _PROGRAM_CACHE["key"] = key
    return _PROGRAM_CACHE["nc"]


def _plane_dev(arr):
    """[T_all, IN] -> [128, 2, JC, T_all/2-per-core] device half-layout.

    j = jc*128 + p; token axis split per-core later. Returns
    [128, JC, T_all] view to be sliced per core then half-split."""
    return np.ascontiguousarray(arr.T.reshape(JC, 128, -1).transpose(1, 0, 2))


def _pack_pair(tab_b):
    """e4m3 [OUT, IN] -> [128p, 2q, 2e, NPASS, 128col] stationary layout."""
    t = tab_b.reshape(NPASS, 128, JC, 128).transpose(3, 2, 0, 1)
    return np.ascontiguousarray(t.reshape(128, 2, 2, NPASS, 128))


def _snap_phi(phi, grid):
    """Snap phi (phi[4]=0 preserved) to fp8-exact values, scaled to ~12."""
    ph = (phi * (12.0 / np.abs(phi).max())).astype(E4NP).astype(np.float64)
    ph[4] = 0.0
    return ph


def _fp8_grid():
    b = np.arange(256, dtype=np.uint8).view(E4NP).astype(np.float64)
    vals = np.unique(b[np.isfinite(b)])
    return vals


def kernel(x: np.ndarray, coeffs: np.ndarray) -> np.ndarray:
    assert x.shape == (8, 2048, IN_F) and coeffs.shape == (OUT_F, IN_F, 12)
    t = np.linspace(0.0, 1.0, 10, dtype=np.float32)  # same knots as reference

    # Segment index via the same float32 comparisons the reference uses.
    xf = np.ascontiguousarray(x.reshape(-1, IN_F))          # [16384, 512]
    seg = np.zeros(xf.shape, dtype=np.int32)
    for m in range(1, 9):
        seg += (xf >= t[m]).astype(np.int32)

    # Table build (see module docstring): scale-free e4m3; phi1/phi2 = top
    # eigenvectors of the m!=4 covariance, fp8-snapped (phi2 values kept
    # distinct for the device is_equal builds); co quantized first
    # (absorbed), residual quantized last, res[4] pinned 0.
    c = coeffs.astype(np.float64)
    F = np.stack(
        [c[:, :, m] + c[:, :, m + 1] + c[:, :, m + 2] for m in range(9)]
    ).reshape(9, -1)                                         # [9, OUT*IN]
    D = F - F[4:5]
    idx = [0, 1, 2, 3, 5, 6, 7, 8]
    C8 = (D[idx] @ D[idx].T) / D.shape[1]
    _, V = np.linalg.eigh(C8)
    grid = _fp8_grid()
    phis = []
    for k in (-1, -2):
        ph = np.zeros(9)
        ph[idx] = V[:, k]
        phis.append(_snap_phi(ph, grid))
    phi1, phi2 = phis
    # ensure phi2 values at m=7,8 are unique (needed for is_equal builds)
    for m in (7, 8):
        others = set(np.delete(phi2, m).tolist())
        if phi2[m] in others:
            gi = int(np.searchsorted(grid, phi2[m]))
            for step in (1, -1, 2, -2, 3, -3):
                cand = grid[(gi + step) % len(grid)]
                if cand not in others and cand != 0.0:
                    phi2[m] = cand
                    break
    assert len(set(phi2[idx].tolist())) == len(idx)

    Phi = np.stack([phi1, phi2], axis=1)                     # [9, 2]
    co = np.linalg.lstsq(Phi[idx], D[idx], rcond=None)[0]    # [2, OUT*IN]

    def q8(v):
        return np.clip(v, -240.0, 240.0).astype(E4NP)

    co1b = q8(co[0].reshape(OUT_F, IN_F))
    co2b = q8(co[1].reshape(OUT_F, IN_F))
    res = D.reshape(9, OUT_F, IN_F) \
        - co1b.astype(np.float64)[None] * phi1[:, None, None] \
        - co2b.astype(np.float64)[None] * phi2[:, None, None]
    Rb = q8(res)
    Rb[4] = 0

    # g tables in unit order: phi1, oh m0..m3, m5 | m6, phi2, m7, m8
    tabs = [co1b] + [Rb[m] for m in (0, 1, 2, 3, 5)] \
        + [Rb[6], co2b, Rb[7], Rb[8]]
    g_dev = np.empty((128, NU, 2, NPASS, 128), dtype=E4NP)
    for ti, tab in enumerate(tabs):
        pk = _pack_pair(tab)
        for q in range(2):
            g_dev[:, 2 * ti + q] = pk[:, q]
    g_dev = np.ascontiguousarray(g_dev)

    base = F[4].reshape(OUT_F, IN_F).sum(axis=1)             # exact fp32
    sb = np.empty((128, NPASS), dtype=np.float32)
    for ob in range(NPASS):
        sb[:, ob] = base[ob * 128:(ob + 1) * 128]

    # Plane bytes via uint8 LUTs over seg (fast).
    planes = np.empty((128, NSHIP, JC, seg.shape[0]), dtype=E4NP)
    for slot, vals in ((0, phi1), (1, phi2)):
        lut = vals.astype(E4NP).view(np.uint8)
        planes[:, slot] = _plane_dev(lut[seg]).view(E4NP)
    for k, m in enumerate(OH_MS):
        lut = np.zeros(9, E4NP)
        lut[m] = 1.0
        planes[:, 2 + k] = _plane_dev(lut.view(np.uint8)[seg]).view(E4NP)

    in_maps = []
    for core in range(N_CORES):
        sl = planes[:, :, :, core * TOK:(core + 1) * TOK]
        in_maps.append(
            {
                "pl": np.ascontiguousarray(sl),
                "g": g_dev,
                "sb": sb,
            }
        )

    phi2_cmp = {7: float(phi2[7]), 8: float(phi2[8])}
    nc = _get_program(phi2_cmp)
    res_ = run_bass_kernel_spmd(nc, in_maps, core_ids=list(range(N_CORES)))
    out = np.stack(
        [
            res_.results[core]["out"].reshape(OUT_F, TOK).T.astype(np.float32)
            for core in range(N_CORES)
        ]
    )
    return np.ascontiguousarray(out)
